# revision 56
# baseline (speedup 1.0000x reference)
"""Single-head attention kernel for Trainium2, SPMD over 8 NeuronCores.

Problem: out = softmax((q@Wq+bq) @ (k@Wk+bk)^T / sqrt(768)) @ (v@Wv+bv)
Shapes: q,k,v [8, 2048, 768] fp32; W* [768, 64]; b* [64].

Strategy: data-parallel over batch (1 batch per core).  Host transposes
q/k/v to partition-major chunked layouts (layout prep only, no FLOPs on
host); q/k cast to fp8-e3m4, v to fp16.  Per core the kernel is a flat
32-unit softmax pipeline; a unit is one (t-block, s-half) score tile
[128, 1024] fp32 in PSUM -> one ScalarE Exp -> fp16 att tile.  ScalarE
is the bottleneck engine (~36 us of Exp), so everything else is
scheduled around keeping it busy:
  - ~52 warmup matmuls run during the DMA ramp so the PE HAM clock
    window latches to 2.4 GHz before real work starts.
  - inputs stream on one sync-queue in need-order, with q/k quarter 0
    split into 256-token halves so the first exp fires early.
  - score matmuls use K=128 lhsT (kiT zero-padded beyond row 64): full
    FWL weight path, and the fp8 qiT moving operand can stream at the
    v3-2x double rate.
  - outputs accumulate per s-QUARTER [128, 512] (rows 0-63 softmax
    denominator via ones-columns in vaug, 64-127 out^T), 4 sequential
    chains double-buffered in 2 PSUM banks; each quarter drains
    (DVE fast-reciprocal + multiply + DMA) as soon as its chain closes.
    This frees 2 banks so score tiles rotate 3-deep in the back half,
    hiding the exp->bank-recycle latency entirely.
  - PE fillers (projections, transposes, output matmuls) are emitted
    BEFORE each unit's score matmuls, paced ~2-4 per slot against DMA
    arrival so the strict-FIFO PE queue never parks on an ungated
    instruction.
"""

import numpy as np
from contextlib import ExitStack

import concourse.bass as bass
import concourse.mybir as mybir
import concourse.tile as tile
from concourse import bacc
from concourse.bass_utils import run_bass_kernel_spmd

E = 768  # n_embd
H = 64  # head size
S = 2048  # sequence length
B = 8  # batch == n_cores
EC = E // 128  # e chunks
TB = S // 128  # t blocks
INV_SQRT_C = float(1.0 / np.sqrt(np.float32(E)))

F16 = mybir.dt.float16
F32 = mybir.dt.float32
F8 = mybir.dt.float8e3  # e3m4: 4 mantissa bits, |x| <= ~15.5

# wa free-dim layout: [wq 6*64 | wk 6*64 | ident 64] -- 1664-byte lines
# (64B-aligned); biases ride a separate tiny first DMA
WA_K = 384
WA_ID = 768
WA_N = WA_ID + 64

N_WARMUP = 48

# Units whose exp runs on the DVE via the fp16 Schraudolph bit-trick:
# bits_i16 = round(s * EXP_A + EXP_B)  ->  reinterpret as fp16 ~= exp(s/sqrt(C))
# (exponent lands in the fp16 exponent field, the fractional part in the
# mantissa; ~1.8% rms relative error on those units' att weights, which
# the softmax normalization averages down to ~0.9% on the output).
# ScalarE stays the bottleneck engine, so every offloaded unit removes
# ~1.1 us from the critical path.
DVE_EXP = frozenset({10, 14, 17, 20, 23, 25, 28, 30})
EXP_A = float(1024.0 / np.log(2.0) / np.sqrt(768.0))
EXP_B = 15 * 1024 - 56.0

_CACHE = {}


def build_program():
    nc = bacc.Bacc(
        "TRN2",
        target_bir_lowering=False,
        debug=False,
        enable_asserts=False,
        num_devices=B,
    )

    q0_d = nc.dram_tensor("q0p", [128, 2, EC, 256], F8, kind="ExternalInput")
    k0_d = nc.dram_tensor("k0p", [128, 2, EC, 256], F8, kind="ExternalInput")
    q_d = nc.dram_tensor("qp", [128, 3, EC, 512], F8, kind="ExternalInput")
    k_d = nc.dram_tensor("kp", [128, 3, EC, 512], F8, kind="ExternalInput")
    v_d = nc.dram_tensor("vp", [128, 4, EC, 512], F16, kind="ExternalInput")
    wa_d = nc.dram_tensor("wa", [128, WA_N], F16, kind="ExternalInput")
    wv_d = nc.dram_tensor("wv", [128, WA_K], F16, kind="ExternalInput")
    bp_d = nc.dram_tensor("bp", [128, 32], F16, kind="ExternalInput")
    outT_d = nc.dram_tensor("outT", [H, S], F16, kind="ExternalOutput")

    with tile.TileContext(nc) as tc, ExitStack() as ctx:
        const = ctx.enter_context(tc.tile_pool(name="const", bufs=1))
        xin = ctx.enter_context(tc.tile_pool(name="xin", bufs=1))
        acts = ctx.enter_context(tc.tile_pool(name="acts", bufs=1))

        wa = const.tile([128, WA_N], F16, tag="wa")
        wv = const.tile([128, WA_K], F16, tag="wv")
        bp = const.tile([128, 32], F16, tag="bp")
        b32 = const.tile([128, 4], F32, tag="b32")
        warm = const.tile([128, 8], F32, tag="warm")
        wu = const.tile([128, 128], F16, tag="wu")

        q0_in = xin.tile([128, 2, EC, 256], F8, tag="q0_in")
        k0_in = xin.tile([128, 2, EC, 256], F8, tag="k0_in")
        q_in = xin.tile([128, 3, EC, 512], F8, tag="q_in")
        k_in = xin.tile([128, 3, EC, 512], F8, tag="k_in")
        v_in = xin.tile([128, 4, EC, 512], F16, tag="v_in")

        # ---- DMA issue: ONE queue (sync), strictly in need-order.  The
        # first 8 transfers cover the ramp; DVE/Scalar setup ops are emitted
        # before the rest so their sem waits don't alias later transfers
        # (Tile reuses DMA semaphores -> false deps).
        nc.sync.dma_start(bp[:], bp_d[:])
        nc.sync.dma_start(wa[:], wa_d[:])
        nc.scalar.dma_start(k0_in[:, 0], k0_d[:, 0])
        nc.sync.dma_start(q0_in[:, 0], q0_d[:, 0])
        nc.scalar.dma_start(q0_in[:, 1], q0_d[:, 1])
        nc.sync.dma_start(q_in[:, 0], q_d[:, 0])  # q1
        nc.scalar.dma_start(k0_in[:, 1], k0_d[:, 1])
        nc.scalar.dma_start(k_in[:, 0], k_d[:, 0])  # k1

        # warm the Exp table on ScalarE while DMAs run
        nc.vector.memset(warm[:], 0.0)
        nc.scalar.activation(
            warm[:], warm[:], mybir.ActivationFunctionType.Exp, scale=1.0
        )
        nc.vector.memset(wu[:], 0.125)

        # qiT fp8 is the scores' moving operand (v3-2x eligible); kiT fp16
        # is the stationary operand.  Rows 64-127 are zero so score matmuls
        # run K=128 (full FWL weight path).
        qiT = acts.tile([128, S], F8, tag="qiT")
        kiT = acts.tile([128, S], F16, tag="kiT")
        viT = acts.tile([64, S], F16, tag="viT")
        vaug = acts.tile([128, S], F16, tag="vaug")
        recip = acts.tile([H, S], F32, tag="recip")
        out_sb = acts.tile([H, S], F16, tag="out_sb")
        # zero the K-padding rows: the slices the ramp's first score matmuls
        # read go on the DVE (cheap, immediate); the bulk goes to the
        # otherwise-idle GpSimd engine so the DVE FIFO stays clear for the
        # first bias-adds
        nc.vector.memset(qiT[64:128, 0:1024], 0.0)
        nc.vector.memset(kiT[64:128, 0:256], 0.0)
        nc.gpsimd.memset(qiT[64:128, 1024:2048], 0.0)
        nc.gpsimd.memset(kiT[64:128, 256:2048], 0.0)
        nc.gpsimd.memset(vaug[:], 1.0)
        # biases fp16 -> fp32 scalars
        nc.vector.tensor_copy(b32[:, 0:3], bp[:, 0:3])

        # remaining input transfers (issued after the DVE setup ops so the
        # setup's sem waits bind to the FIRST use of each DMA semaphore)
        nc.sync.dma_start(k_in[:, 1], k_d[:, 1])  # k2
        nc.gpsimd.dma_start(q_in[:, 1], q_d[:, 1])  # q2
        nc.sync.dma_start(wv[:], wv_d[:])
        nc.sync.dma_start(v_in[:, 0], v_d[:, 0])
        nc.gpsimd.dma_start(v_in[:, 1], v_d[:, 1])
        nc.sync.dma_start(k_in[:, 2], k_d[:, 2])  # k3
        nc.gpsimd.dma_start(q_in[:, 2], q_d[:, 2])  # q3
        nc.sync.dma_start(v_in[:, 2], v_d[:, 2])
        nc.gpsimd.dma_start(v_in[:, 3], v_d[:, 3])

        attp = ctx.enter_context(tc.tile_pool(name="attp", bufs=16))
        attTs = [
            attp.tile([128, S], F16, tag="attT", name=f"attT{i}") for i in range(TB)
        ]

        def wq_ap(c):
            return wa[:, c * 64 : (c + 1) * 64]

        def wk_ap(c):
            return wa[:, WA_K + c * 64 : WA_K + (c + 1) * 64]

        def wv_ap(c):
            return wv[:, c * 64 : (c + 1) * 64]

        id_ap = wa[0:64, WA_ID : WA_ID + 64]

        # score tiles: 2 double-buffered [128, 1024] fp32 (4 banks); a third
        # rotation tile (ps_x) joins once the projection pool closes
        ps = ctx.enter_context(tc.tile_pool(name="ps", bufs=2, space="PSUM"))
        # output accumulators: per-s-quarter [128, 512], double-buffered.
        # The warmup/keepalive tile borrows a buffer of this pool -- its
        # bank recycles into the q1-quarter accumulator long after the last
        # keepalive matmul has run.
        poq = ctx.enter_context(tc.tile_pool(name="poq", bufs=2, space="PSUM"))
        po_q = [None] * 4
        ps_x_tile = [None]

        def sc_unit(u):
            tb, h = u % 16, u // 16
            if u >= 20 and (u - 20) % 3 == 2:
                pt = ps_x_tile[0].tile(
                    [128, 1024], F32, tag="psx", name=f"sc{tb}_{h}"
                )
            else:
                pt = ps.tile([128, 1024], F32, tag="ps", name=f"sc{tb}_{h}")
            for j in range(2):
                nc.tensor.matmul(
                    pt[:, j * 512 : (j + 1) * 512],
                    lhsT=kiT[:, tb * 128 : (tb + 1) * 128],
                    rhs=qiT[:, h * 1024 + j * 512 : h * 1024 + (j + 1) * 512],
                    start=True,
                    stop=True,
                )
            return pt

        def exp_unit(u, pt):
            tb, h = u % 16, u // 16
            sl = slice(h * 1024, (h + 1) * 1024)
            if u in DVE_EXP:
                nc.vector.tensor_scalar(
                    attTs[tb][:, sl].bitcast(mybir.dt.int16),
                    pt[:],
                    EXP_A,
                    EXP_B,
                    op0=mybir.AluOpType.mult,
                    op1=mybir.AluOpType.add,
                )
            else:
                nc.scalar.activation(
                    attTs[tb][:, sl],
                    pt[:],
                    mybir.ActivationFunctionType.Exp,
                    scale=INV_SQRT_C,
                )

        def out_q(tb, sq, first=False, last=False):
            # accumulate t-block tb into s-quarter sq: rows 0-63 denominator,
            # 64-127 out^T
            if first:
                po_q[sq] = poq.tile([128, 512], F32, tag="poq", name=f"po{sq}")
            nc.tensor.matmul(
                po_q[sq][:, :],
                lhsT=vaug[:, tb * 128 : (tb + 1) * 128],
                rhs=attTs[tb][:, sq * 512 : (sq + 1) * 512],
                start=first,
                stop=last,
            )

        def drain_q(sq):
            sl = slice(sq * 512, (sq + 1) * 512)
            nc.vector.reciprocal_approx_fast(recip[:, sl], po_q[sq][0:64, :])
            nc.vector.tensor_tensor(
                out_sb[:, sl],
                po_q[sq][64:128, :],
                recip[:, sl],
                op=mybir.AluOpType.mult,
            )
            eng = nc.sync if sq < 2 else nc.scalar
            eng.dma_start(outT_d[:, sl], out_sb[:, sl])

        with tc.tile_pool(name="pp", bufs=2, space="PSUM") as pp:
            # ---- PE warmup: latch the HAM activity window to full clock ----
            pwu = poq.tile([128, 512], F32, tag="poq", name="pwu")
            for _ in range(N_WARMUP):
                nc.tensor.matmul(
                    pwu[:, 0:128], lhsT=wu[:], rhs=wu[:], start=True, stop=True
                )

            def proj_qk0(hlf):
                # quarter 0 half hlf (256 tokens): q -> PE cols 0-63,
                # k -> cols 64-127, concurrent
                pj = pp.tile([128, 512], F32, tag="pp", name=f"pqk0_{hlf}")
                for c in range(EC):
                    nc.tensor.matmul(
                        pj[0:64, 0:256],
                        lhsT=wq_ap(c),
                        rhs=q0_in[:, hlf, c],
                        start=(c == 0),
                        stop=(c == EC - 1),
                        skip_group_check=True,
                    )
                    nc.tensor.matmul(
                        pj[64:128, 0:256],
                        lhsT=wk_ap(c),
                        rhs=k0_in[:, hlf, c],
                        start=(c == 0),
                        stop=(c == EC - 1),
                        skip_group_check=True,
                    )
                sl = slice(hlf * 256, (hlf + 1) * 256)
                # q-side add on DVE, k-side on the (ramp-idle) ScalarE so the
                # two run concurrently -- this chain gates the first exp
                nc.vector.tensor_scalar_add(
                    qiT[0:64, sl], pj[0:64, 0:256], b32[0:64, 0:1]
                )
                nc.scalar.activation(
                    kiT[0:64, sl],
                    pj[64:128, 0:256],
                    mybir.ActivationFunctionType.Identity,
                    bias=b32[0:64, 1:2],
                    scale=1.0,
                )

            def proj_one(t, dst, src_in, jq):
                # unpaired projection of one 512-token quarter
                pj = pp.tile([128, 512], F32, tag="pp", name=f"p1_{t}_{jq}")
                w_ap = (wq_ap, wk_ap, wv_ap)[t]
                for c in range(EC):
                    nc.tensor.matmul(
                        pj[0:64, :],
                        lhsT=w_ap(c),
                        rhs=src_in[:, jq, c],
                        start=(c == 0),
                        stop=(c == EC - 1),
                    )
                sl = (
                    slice((jq + 1) * 512, (jq + 2) * 512)
                    if t < 2
                    else slice(jq * 512, (jq + 1) * 512)
                )
                nc.vector.tensor_scalar_add(
                    dst[0:64, sl] if t < 2 else dst[:, sl],
                    pj[0:64, :],
                    b32[0:64, t : t + 1],
                )

            _qk_half = {}

            def proj_qk_a(jq):
                pj = pp.tile([128, 512], F32, tag="pp", name=f"pqk{jq}")
                _qk_half[jq] = pj
                for c in range(3):
                    nc.tensor.matmul(
                        pj[0:64, :], lhsT=wq_ap(c), rhs=q_in[:, jq, c],
                        start=(c == 0), stop=False, skip_group_check=True,
                    )
                    nc.tensor.matmul(
                        pj[64:128, :], lhsT=wk_ap(c), rhs=k_in[:, jq, c],
                        start=(c == 0), stop=False, skip_group_check=True,
                    )

            def proj_qk_b(jq):
                pj = _qk_half[jq]
                for c in range(3, EC):
                    nc.tensor.matmul(
                        pj[0:64, :], lhsT=wq_ap(c), rhs=q_in[:, jq, c],
                        start=False, stop=(c == EC - 1), skip_group_check=True,
                    )
                    nc.tensor.matmul(
                        pj[64:128, :], lhsT=wk_ap(c), rhs=k_in[:, jq, c],
                        start=False, stop=(c == EC - 1), skip_group_check=True,
                    )
                sl = slice((jq + 1) * 512, (jq + 2) * 512)
                nc.vector.tensor_scalar_add(qiT[0:64, sl], pj[0:64, :], b32[0:64, 0:1])
                nc.vector.tensor_scalar_add(kiT[0:64, sl], pj[64:128, :], b32[0:64, 1:2])

            _pv_half = {}

            def proj_v_a(jpair):
                pj = pp.tile([128, 512], F32, tag="pp", name=f"pv{jpair}")
                _pv_half[jpair] = pj
                j0, j1 = 2 * jpair, 2 * jpair + 1
                for c in range(3):
                    nc.tensor.matmul(
                        pj[0:64, :], lhsT=wv_ap(c), rhs=v_in[:, j0, c],
                        start=(c == 0), stop=False, skip_group_check=True,
                    )
                    nc.tensor.matmul(
                        pj[64:128, :], lhsT=wv_ap(c), rhs=v_in[:, j1, c],
                        start=(c == 0), stop=False, skip_group_check=True,
                    )

            def proj_v_b(jpair):
                pj = _pv_half[jpair]
                j0, j1 = 2 * jpair, 2 * jpair + 1
                for c in range(3, EC):
                    nc.tensor.matmul(
                        pj[0:64, :], lhsT=wv_ap(c), rhs=v_in[:, j0, c],
                        start=False, stop=(c == EC - 1), skip_group_check=True,
                    )
                    nc.tensor.matmul(
                        pj[64:128, :], lhsT=wv_ap(c), rhs=v_in[:, j1, c],
                        start=False, stop=(c == EC - 1), skip_group_check=True,
                    )
                nc.vector.tensor_scalar_add(
                    viT[:, j0 * 512 : (j0 + 1) * 512], pj[0:64, :], b32[0:64, 2:3]
                )
                nc.vector.tensor_scalar_add(
                    viT[:, j1 * 512 : (j1 + 1) * 512], pj[64:128, :], b32[0:64, 2:3]
                )

            def transp(g):
                # viT [64, 512] quarter g -> vi blocks [128, 64] into vaug
                # cols 64-127 via PE transpose.  vaug ones-cols come from the
                # memset filler.
                tr = pp.tile([128, 512], F16, tag="pp", name=f"tr{g}")
                for i in range(4):
                    tb = g * 4 + i
                    nc.tensor.transpose(
                        tr[:, i * 64 : (i + 1) * 64],
                        viT[:, tb * 128 : (tb + 1) * 128],
                        id_ap,
                    )
                dst_ap = vaug[:, g * 512 : (g + 1) * 512].rearrange(
                    "p (t c) -> p t c", c=128
                )[:, :, 64:128]
                src_ap = tr[:, 0:256].rearrange("p (t c) -> p t c", c=H)
                nc.vector.tensor_copy(dst_ap, src_ap)

            def keepalive(n):
                # tiny matmuls that keep the PE HAM activity window busy
                # through DMA-gated stretches of the ramp (a >3.4us PE idle
                # re-throttles the clock to 1.2 GHz)
                def f():
                    for _ in range(n):
                        nc.tensor.matmul(
                            pwu[:, 128:256], lhsT=wu[:], rhs=wu[:],
                            start=True, stop=True,
                        )
                return f

            def proj_h0b(t):
                # unpaired projection of q/k quarter-0 half 1 (256 tokens)
                pj = pp.tile([128, 512], F32, tag="pp", name=f"ph0b{t}")
                src = (q0_in, k0_in)[t]
                w_ap = (wq_ap, wk_ap)[t]
                dst = (qiT, kiT)[t]
                for c in range(EC):
                    nc.tensor.matmul(
                        pj[0:64, 0:256],
                        lhsT=w_ap(c),
                        rhs=src[:, 1, c],
                        start=(c == 0),
                        stop=(c == EC - 1),
                    )
                nc.vector.tensor_scalar_add(
                    dst[0:64, 256:512], pj[0:64, 0:256], b32[0:64, t : t + 1]
                )

            # ---- ramp: quarter-0 halves + unit 0 in 3 chunks, with
            # keepalive batches filling DMA-bound PE idle ----
            proj_qk0(0)
            pt0 = ps.tile([128, 1024], F32, tag="ps", name="sc0_0")

            def sc0(j0, j1):
                nc.tensor.matmul(
                    pt0[:, j0:j1],
                    lhsT=kiT[:, 0:128],
                    rhs=qiT[:, j0:j1],
                    start=True,
                    stop=True,
                )
                nc.scalar.activation(
                    attTs[0][:, j0:j1],
                    pt0[:, j0:j1],
                    mybir.ActivationFunctionType.Exp,
                    scale=INV_SQRT_C,
                )

            sc0(0, 256)
            proj_h0b(0)  # q0b unpaired
            sc0(256, 512)
            proj_one(0, qiT, q_in, 0)  # q1 unpaired: gates unit-0 chunk 3
            sc0(512, 1024)

            F = {
                2: [lambda: proj_h0b(1), lambda: proj_one(1, kiT, k_in, 0)],  # k0b, k1
                5: [lambda: proj_qk_a(1)],  # q2+k2
                6: [lambda: proj_qk_b(1)],
                8: [lambda: proj_v_a(0)],  # v0+v1
                9: [lambda: proj_v_b(0)],
                10: [lambda: proj_qk_a(2)],  # q3+k3
                11: [lambda: proj_qk_b(2)],
                12: [lambda: transp(0)],
                13: [lambda: transp(1), lambda: out_q(0, 0, first=True),
                     lambda: out_q(1, 0)],
                14: [lambda: out_q(2, 0), lambda: out_q(3, 0),
                     lambda: out_q(0, 1, first=True)],
                15: [lambda: proj_v_a(1), lambda: out_q(1, 1),
                     lambda: out_q(5, 1)],  # v2+v3
                16: [lambda: proj_v_b(1), lambda: out_q(4, 0)],
                17: [lambda: out_q(2, 1), lambda: out_q(3, 1), lambda: out_q(5, 0)],
                18: [lambda: transp(2), lambda: out_q(6, 0), lambda: out_q(7, 0)],
                19: [lambda: transp(3), lambda: out_q(4, 1), lambda: out_q(8, 0)],
                20: [lambda: out_q(9, 0), lambda: out_q(6, 1), lambda: out_q(7, 1)],
                21: [lambda: out_q(10, 0), lambda: out_q(11, 0), lambda: out_q(8, 1)],
                22: [lambda: out_q(12, 0), lambda: out_q(13, 0), lambda: out_q(9, 1)],
                23: [lambda: out_q(14, 0), lambda: out_q(15, 0, last=True),
                     lambda: out_q(10, 1)],
                24: [lambda: drain_q(0), lambda: out_q(11, 1), lambda: out_q(12, 1)],
                25: [lambda: out_q(13, 1), lambda: out_q(14, 1),
                     lambda: out_q(15, 1, last=True)],
                26: [lambda: drain_q(1), lambda: out_q(0, 2, first=True),
                     lambda: out_q(1, 2), lambda: out_q(0, 3, first=True)],
                27: [lambda: out_q(2, 2), lambda: out_q(3, 2), lambda: out_q(1, 3),
                     lambda: out_q(2, 3)],
                28: [lambda: out_q(4, 2), lambda: out_q(5, 2), lambda: out_q(3, 3),
                     lambda: out_q(4, 3)],
                29: [lambda: out_q(6, 2), lambda: out_q(7, 2), lambda: out_q(5, 3),
                     lambda: out_q(6, 3)],
                30: [lambda: out_q(8, 2), lambda: out_q(9, 2), lambda: out_q(7, 3),
                     lambda: out_q(8, 3)],
                31: [lambda: out_q(10, 2), lambda: out_q(11, 2), lambda: out_q(9, 3),
                     lambda: out_q(10, 3)],
            }

            def emit_unit(u):
                for f in F.get(u, []):
                    f()
                pt = sc_unit(u)
                exp_unit(u, pt)

            for u in range(1, 20):
                emit_unit(u)

        # pp closed: its banks host the third score rotation tile
        with tc.tile_pool(name="psx", bufs=1, space="PSUM") as psx:
            ps_x_tile[0] = psx
            for u in range(20, 32):
                emit_unit(u)

            # ---- tail: interleave the two remaining chains so q2 closes
            # (and starts draining) while q3's last matmuls still run ----
            out_q(12, 2)
            out_q(11, 3)
            out_q(13, 2)
            out_q(12, 3)
            out_q(14, 2)
            out_q(13, 3)
            out_q(15, 2, last=True)
            out_q(14, 3)
            drain_q(2)
            out_q(15, 3, last=True)
            drain_q(3)

    nc.compile()
    return nc


def _prep_inputs(q, k, v, Wq, bq, Wk, bk, Wv, bv):
    """Host-side layout prep: per-batch transpose + dtype cast + packing."""
    import ml_dtypes

    wa = np.zeros((128, WA_N), dtype=np.float16)
    for t, W in enumerate((Wq, Wk)):
        wa[:, t * WA_K : (t + 1) * WA_K] = (
            np.asarray(W, np.float16).reshape(EC, 128, 64).transpose(1, 0, 2).reshape(128, WA_K)
        )
    wa[0:64, WA_ID : WA_ID + 64] = np.eye(64, dtype=np.float16)
    bpk = np.zeros((128, 32), dtype=np.float16)
    for i, b in enumerate((bq, bk, bv)):
        bpk[:, i] = np.tile(np.asarray(b, dtype=np.float16).reshape(64), 2)
    wvp = np.ascontiguousarray(
        np.asarray(Wv, np.float16).reshape(EC, 128, 64).transpose(1, 0, 2).reshape(128, WA_K)
    )

    def pack_first(x, dt):
        xT = np.asarray(x[:512], dtype=dt).T  # [768, 512]
        return np.ascontiguousarray(xT.reshape(EC, 128, 2, 256).transpose(1, 2, 0, 3))

    def pack_rest(x, dt):
        xT = np.asarray(x[512:], dtype=dt).T  # [768, 1536]
        return np.ascontiguousarray(xT.reshape(EC, 128, 3, 512).transpose(1, 2, 0, 3))

    def pack_full(x, dt):
        xT = np.asarray(x, dtype=dt).T  # [768, 2048]
        return np.ascontiguousarray(xT.reshape(EC, 128, 4, 512).transpose(1, 2, 0, 3))

    f8 = ml_dtypes.float8_e3m4
    in_maps = []
    for i in range(B):
        m = {
            "q0p": pack_first(q[i], f8),
            "k0p": pack_first(k[i], f8),
            "qp": pack_rest(q[i], f8),
            "kp": pack_rest(k[i], f8),
            "vp": pack_full(v[i], np.float16),
            "wa": wa,
            "wv": wvp,
            "bp": bpk,
        }
        in_maps.append(m)
    return in_maps


def run(trace=False, **inputs):
    """Build (cached), run on 8 cores, gather. Returns (out, BassKernelResults)."""
    if "nc" not in _CACHE:
        _CACHE["nc"] = build_program()
    nc = _CACHE["nc"]
    in_maps = _prep_inputs(**{k2: np.asarray(v2) for k2, v2 in inputs.items()})
    res = run_bass_kernel_spmd(nc, in_maps, list(range(B)), trace=trace)
    out = np.stack([np.ascontiguousarray(res.results[i]["outT"].T) for i in range(B)])
    return out.astype(np.float32), res


def kernel(**inputs) -> np.ndarray:
    out, _ = run(trace=False, **inputs)
    return out


# revision 57
# speedup vs baseline: 1.0390x; 1.0390x over previous
"""Single-head attention kernel for Trainium2, SPMD over 8 NeuronCores.

Problem: out = softmax((q@Wq+bq) @ (k@Wk+bk)^T / sqrt(768)) @ (v@Wv+bv)
Shapes: q,k,v [8, 2048, 768] fp32; W* [768, 64]; b* [64].

Strategy: data-parallel over batch (1 batch per core).  Host transposes
q/k/v to partition-major chunked layouts (layout prep only, no FLOPs on
host); q/k cast to fp8-e3m4, v to fp16.  Per core the kernel is a flat
32-unit softmax pipeline; a unit is one (t-block, s-half) score tile
[128, 1024] fp32 in PSUM -> one ScalarE Exp -> fp16 att tile.  ScalarE
is the bottleneck engine (~36 us of Exp), so everything else is
scheduled around keeping it busy:
  - ~52 warmup matmuls run during the DMA ramp so the PE HAM clock
    window latches to 2.4 GHz before real work starts.
  - inputs stream on one sync-queue in need-order, with q/k quarter 0
    split into 256-token halves so the first exp fires early.
  - score matmuls use K=128 lhsT (kiT zero-padded beyond row 64): full
    FWL weight path, and the fp8 qiT moving operand can stream at the
    v3-2x double rate.
  - outputs accumulate per s-QUARTER [128, 512] (rows 0-63 softmax
    denominator via ones-columns in vaug, 64-127 out^T), 4 sequential
    chains double-buffered in 2 PSUM banks; each quarter drains
    (DVE fast-reciprocal + multiply + DMA) as soon as its chain closes.
    This frees 2 banks so score tiles rotate 3-deep in the back half,
    hiding the exp->bank-recycle latency entirely.
  - PE fillers (projections, transposes, output matmuls) are emitted
    BEFORE each unit's score matmuls, paced ~2-4 per slot against DMA
    arrival so the strict-FIFO PE queue never parks on an ungated
    instruction.
"""

import numpy as np
from contextlib import ExitStack

import concourse.bass as bass
import concourse.mybir as mybir
import concourse.tile as tile
from concourse import bacc
from concourse.bass_utils import run_bass_kernel_spmd

E = 768  # n_embd
H = 64  # head size
S = 2048  # sequence length
B = 8  # batch == n_cores
EC = E // 128  # e chunks
TB = S // 128  # t blocks
INV_SQRT_C = float(1.0 / np.sqrt(np.float32(E)))

F16 = mybir.dt.float16
F32 = mybir.dt.float32
F8 = mybir.dt.float8e3  # e3m4: 4 mantissa bits, |x| <= ~15.5

# wa free-dim layout: [wq 6*64 | wk 6*64 | ident 64] -- 1664-byte lines
# (64B-aligned); biases ride a separate tiny first DMA
WA_K = 384
WA_ID = 768
WA_N = WA_ID + 64

N_WARMUP = 48

# Units whose exp runs on the DVE via the fp16 Schraudolph bit-trick:
# bits_i16 = round(s * EXP_A + EXP_B)  ->  reinterpret as fp16 ~= exp(s/sqrt(C))
# (exponent lands in the fp16 exponent field, the fractional part in the
# mantissa; ~1.8% rms relative error on those units' att weights, which
# the softmax normalization averages down to ~0.9% on the output).
# ScalarE stays the bottleneck engine, so every offloaded unit removes
# ~1.1 us from the critical path.
DVE_EXP = frozenset({10, 14, 17, 20, 23, 25, 28, 30})
EXP_A = float(1024.0 / np.log(2.0) / np.sqrt(768.0))
EXP_B = 15 * 1024 - 56.0

_CACHE = {}


def build_program():
    nc = bacc.Bacc(
        "TRN2",
        target_bir_lowering=False,
        debug=False,
        enable_asserts=False,
        num_devices=B,
    )

    q0_d = nc.dram_tensor("q0p", [128, 2, EC, 256], F8, kind="ExternalInput")
    k0_d = nc.dram_tensor("k0p", [128, 2, EC, 256], F8, kind="ExternalInput")
    q_d = nc.dram_tensor("qp", [128, 3, EC, 512], F8, kind="ExternalInput")
    k_d = nc.dram_tensor("kp", [128, 3, EC, 512], F8, kind="ExternalInput")
    v_d = nc.dram_tensor("vp", [128, 4, EC, 512], F16, kind="ExternalInput")
    wa_d = nc.dram_tensor("wa", [128, WA_N], F16, kind="ExternalInput")
    wv_d = nc.dram_tensor("wv", [128, WA_K], F16, kind="ExternalInput")
    bp_d = nc.dram_tensor("bp", [128, 32], F16, kind="ExternalInput")
    outT_d = nc.dram_tensor("outT", [H, S], F16, kind="ExternalOutput")

    with tile.TileContext(nc) as tc, ExitStack() as ctx:
        const = ctx.enter_context(tc.tile_pool(name="const", bufs=1))
        xin = ctx.enter_context(tc.tile_pool(name="xin", bufs=1))
        acts = ctx.enter_context(tc.tile_pool(name="acts", bufs=1))

        wa = const.tile([128, WA_N], F16, tag="wa")
        wv = const.tile([128, WA_K], F16, tag="wv")
        bp = const.tile([128, 32], F16, tag="bp")
        b32 = const.tile([128, 4], F32, tag="b32")
        warm = const.tile([128, 8], F32, tag="warm")
        wu = const.tile([128, 128], F16, tag="wu")

        q0_in = xin.tile([128, 2, EC, 256], F8, tag="q0_in")
        k0_in = xin.tile([128, 2, EC, 256], F8, tag="k0_in")
        q_in = xin.tile([128, 3, EC, 512], F8, tag="q_in")
        k_in = xin.tile([128, 3, EC, 512], F8, tag="k_in")
        v_in = xin.tile([128, 4, EC, 512], F16, tag="v_in")

        # ---- DMA issue: ONE queue (sync), strictly in need-order.  The
        # first 8 transfers cover the ramp; DVE/Scalar setup ops are emitted
        # before the rest so their sem waits don't alias later transfers
        # (Tile reuses DMA semaphores -> false deps).
        nc.sync.dma_start(bp[:], bp_d[:])
        nc.sync.dma_start(wa[:], wa_d[:])
        nc.sync.dma_start(k0_in[:, 0], k0_d[:, 0])
        nc.sync.dma_start(q0_in[:, 0], q0_d[:, 0])
        nc.sync.dma_start(q0_in[:, 1], q0_d[:, 1])
        nc.sync.dma_start(q_in[:, 0], q_d[:, 0])  # q1
        nc.sync.dma_start(k0_in[:, 1], k0_d[:, 1])
        nc.sync.dma_start(k_in[:, 0], k_d[:, 0])  # k1

        # warm the Exp table on ScalarE while DMAs run
        nc.vector.memset(warm[:], 0.0)
        nc.scalar.activation(
            warm[:], warm[:], mybir.ActivationFunctionType.Exp, scale=1.0
        )
        nc.vector.memset(wu[:], 0.125)

        # qiT fp8 is the scores' moving operand (v3-2x eligible); kiT fp16
        # is the stationary operand.  Rows 64-127 are zero so score matmuls
        # run K=128 (full FWL weight path).
        qiT = acts.tile([128, S], F8, tag="qiT")
        kiT = acts.tile([128, S], F16, tag="kiT")
        viT = acts.tile([64, S], F16, tag="viT")
        vaug = acts.tile([128, S], F16, tag="vaug")
        recip = acts.tile([H, S], F32, tag="recip")
        out_sb = acts.tile([H, S], F16, tag="out_sb")
        # zero the K-padding rows: the slices the ramp's first score matmuls
        # read go on the DVE (cheap, immediate); the bulk goes to the
        # otherwise-idle GpSimd engine so the DVE FIFO stays clear for the
        # first bias-adds
        nc.vector.memset(qiT[64:128, 0:1024], 0.0)
        nc.vector.memset(kiT[64:128, 0:256], 0.0)
        nc.gpsimd.memset(qiT[64:128, 1024:2048], 0.0)
        nc.gpsimd.memset(kiT[64:128, 256:2048], 0.0)
        nc.gpsimd.memset(vaug[:], 1.0)
        # biases fp16 -> fp32 scalars
        nc.vector.tensor_copy(b32[:, 0:3], bp[:, 0:3])

        # remaining input transfers (issued after the DVE setup ops so the
        # setup's sem waits bind to the FIRST use of each DMA semaphore)
        nc.sync.dma_start(k_in[:, 1], k_d[:, 1])  # k2
        nc.sync.dma_start(q_in[:, 1], q_d[:, 1])  # q2
        nc.sync.dma_start(wv[:], wv_d[:])
        nc.sync.dma_start(v_in[:, 0], v_d[:, 0])
        nc.sync.dma_start(v_in[:, 1], v_d[:, 1])
        nc.sync.dma_start(k_in[:, 2], k_d[:, 2])  # k3
        nc.sync.dma_start(q_in[:, 2], q_d[:, 2])  # q3
        nc.sync.dma_start(v_in[:, 2], v_d[:, 2])
        nc.sync.dma_start(v_in[:, 3], v_d[:, 3])

        attp = ctx.enter_context(tc.tile_pool(name="attp", bufs=16))
        attTs = [
            attp.tile([128, S], F16, tag="attT", name=f"attT{i}") for i in range(TB)
        ]

        def wq_ap(c):
            return wa[:, c * 64 : (c + 1) * 64]

        def wk_ap(c):
            return wa[:, WA_K + c * 64 : WA_K + (c + 1) * 64]

        def wv_ap(c):
            return wv[:, c * 64 : (c + 1) * 64]

        id_ap = wa[0:64, WA_ID : WA_ID + 64]

        # score tiles: 2 double-buffered [128, 1024] fp32 (4 banks); a third
        # rotation tile (ps_x) joins once the projection pool closes
        ps = ctx.enter_context(tc.tile_pool(name="ps", bufs=2, space="PSUM"))
        # output accumulators: per-s-quarter [128, 512], double-buffered.
        # The warmup/keepalive tile borrows a buffer of this pool -- its
        # bank recycles into the q1-quarter accumulator long after the last
        # keepalive matmul has run.
        poq = ctx.enter_context(tc.tile_pool(name="poq", bufs=2, space="PSUM"))
        po_q = [None] * 4
        ps_x_tile = [None]

        def sc_unit(u):
            tb, h = u % 16, u // 16
            if u >= 20 and (u - 20) % 3 == 2:
                pt = ps_x_tile[0].tile(
                    [128, 1024], F32, tag="psx", name=f"sc{tb}_{h}"
                )
            else:
                pt = ps.tile([128, 1024], F32, tag="ps", name=f"sc{tb}_{h}")
            for j in range(2):
                nc.tensor.matmul(
                    pt[:, j * 512 : (j + 1) * 512],
                    lhsT=kiT[:, tb * 128 : (tb + 1) * 128],
                    rhs=qiT[:, h * 1024 + j * 512 : h * 1024 + (j + 1) * 512],
                    start=True,
                    stop=True,
                )
            return pt

        def exp_unit(u, pt):
            tb, h = u % 16, u // 16
            sl = slice(h * 1024, (h + 1) * 1024)
            if u in DVE_EXP:
                nc.vector.tensor_scalar(
                    attTs[tb][:, sl].bitcast(mybir.dt.int16),
                    pt[:],
                    EXP_A,
                    EXP_B,
                    op0=mybir.AluOpType.mult,
                    op1=mybir.AluOpType.add,
                )
            else:
                nc.scalar.activation(
                    attTs[tb][:, sl],
                    pt[:],
                    mybir.ActivationFunctionType.Exp,
                    scale=INV_SQRT_C,
                )

        def out_q(tb, sq, first=False, last=False):
            # accumulate t-block tb into s-quarter sq: rows 0-63 denominator,
            # 64-127 out^T
            if first:
                po_q[sq] = poq.tile([128, 512], F32, tag="poq", name=f"po{sq}")
            nc.tensor.matmul(
                po_q[sq][:, :],
                lhsT=vaug[:, tb * 128 : (tb + 1) * 128],
                rhs=attTs[tb][:, sq * 512 : (sq + 1) * 512],
                start=first,
                stop=last,
            )

        def drain_q(sq):
            sl = slice(sq * 512, (sq + 1) * 512)
            nc.vector.reciprocal_approx_fast(recip[:, sl], po_q[sq][0:64, :])
            nc.vector.tensor_tensor(
                out_sb[:, sl],
                po_q[sq][64:128, :],
                recip[:, sl],
                op=mybir.AluOpType.mult,
            )
            eng = nc.sync if sq < 2 else nc.scalar
            eng.dma_start(outT_d[:, sl], out_sb[:, sl])

        with tc.tile_pool(name="pp", bufs=2, space="PSUM") as pp:
            # ---- PE warmup: latch the HAM activity window to full clock ----
            pwu = poq.tile([128, 512], F32, tag="poq", name="pwu")
            for _ in range(N_WARMUP):
                nc.tensor.matmul(
                    pwu[:, 0:128], lhsT=wu[:], rhs=wu[:], start=True, stop=True
                )

            def proj_qk0(hlf):
                # quarter 0 half hlf (256 tokens): q -> PE cols 0-63,
                # k -> cols 64-127, concurrent
                pj = pp.tile([128, 512], F32, tag="pp", name=f"pqk0_{hlf}")
                for c in range(EC):
                    nc.tensor.matmul(
                        pj[0:64, 0:256],
                        lhsT=wq_ap(c),
                        rhs=q0_in[:, hlf, c],
                        start=(c == 0),
                        stop=(c == EC - 1),
                        skip_group_check=True,
                    )
                    nc.tensor.matmul(
                        pj[64:128, 0:256],
                        lhsT=wk_ap(c),
                        rhs=k0_in[:, hlf, c],
                        start=(c == 0),
                        stop=(c == EC - 1),
                        skip_group_check=True,
                    )
                sl = slice(hlf * 256, (hlf + 1) * 256)
                # q-side add on DVE, k-side on the (ramp-idle) ScalarE so the
                # two run concurrently -- this chain gates the first exp
                nc.vector.tensor_scalar_add(
                    qiT[0:64, sl], pj[0:64, 0:256], b32[0:64, 0:1]
                )
                nc.scalar.activation(
                    kiT[0:64, sl],
                    pj[64:128, 0:256],
                    mybir.ActivationFunctionType.Identity,
                    bias=b32[0:64, 1:2],
                    scale=1.0,
                )

            def proj_one(t, dst, src_in, jq):
                # unpaired projection of one 512-token quarter
                pj = pp.tile([128, 512], F32, tag="pp", name=f"p1_{t}_{jq}")
                w_ap = (wq_ap, wk_ap, wv_ap)[t]
                for c in range(EC):
                    nc.tensor.matmul(
                        pj[0:64, :],
                        lhsT=w_ap(c),
                        rhs=src_in[:, jq, c],
                        start=(c == 0),
                        stop=(c == EC - 1),
                    )
                sl = (
                    slice((jq + 1) * 512, (jq + 2) * 512)
                    if t < 2
                    else slice(jq * 512, (jq + 1) * 512)
                )
                nc.vector.tensor_scalar_add(
                    dst[0:64, sl] if t < 2 else dst[:, sl],
                    pj[0:64, :],
                    b32[0:64, t : t + 1],
                )

            _qk_half = {}

            def proj_qk_a(jq):
                pj = pp.tile([128, 512], F32, tag="pp", name=f"pqk{jq}")
                _qk_half[jq] = pj
                for c in range(3):
                    nc.tensor.matmul(
                        pj[0:64, :], lhsT=wq_ap(c), rhs=q_in[:, jq, c],
                        start=(c == 0), stop=False, skip_group_check=True,
                    )
                    nc.tensor.matmul(
                        pj[64:128, :], lhsT=wk_ap(c), rhs=k_in[:, jq, c],
                        start=(c == 0), stop=False, skip_group_check=True,
                    )

            def proj_qk_b(jq):
                pj = _qk_half[jq]
                for c in range(3, EC):
                    nc.tensor.matmul(
                        pj[0:64, :], lhsT=wq_ap(c), rhs=q_in[:, jq, c],
                        start=False, stop=(c == EC - 1), skip_group_check=True,
                    )
                    nc.tensor.matmul(
                        pj[64:128, :], lhsT=wk_ap(c), rhs=k_in[:, jq, c],
                        start=False, stop=(c == EC - 1), skip_group_check=True,
                    )
                sl = slice((jq + 1) * 512, (jq + 2) * 512)
                nc.vector.tensor_scalar_add(qiT[0:64, sl], pj[0:64, :], b32[0:64, 0:1])
                nc.vector.tensor_scalar_add(kiT[0:64, sl], pj[64:128, :], b32[0:64, 1:2])

            _pv_half = {}

            def proj_v_a(jpair):
                pj = pp.tile([128, 512], F32, tag="pp", name=f"pv{jpair}")
                _pv_half[jpair] = pj
                j0, j1 = 2 * jpair, 2 * jpair + 1
                for c in range(3):
                    nc.tensor.matmul(
                        pj[0:64, :], lhsT=wv_ap(c), rhs=v_in[:, j0, c],
                        start=(c == 0), stop=False, skip_group_check=True,
                    )
                    nc.tensor.matmul(
                        pj[64:128, :], lhsT=wv_ap(c), rhs=v_in[:, j1, c],
                        start=(c == 0), stop=False, skip_group_check=True,
                    )

            def proj_v_b(jpair):
                pj = _pv_half[jpair]
                j0, j1 = 2 * jpair, 2 * jpair + 1
                for c in range(3, EC):
                    nc.tensor.matmul(
                        pj[0:64, :], lhsT=wv_ap(c), rhs=v_in[:, j0, c],
                        start=False, stop=(c == EC - 1), skip_group_check=True,
                    )
                    nc.tensor.matmul(
                        pj[64:128, :], lhsT=wv_ap(c), rhs=v_in[:, j1, c],
                        start=False, stop=(c == EC - 1), skip_group_check=True,
                    )
                nc.vector.tensor_scalar_add(
                    viT[:, j0 * 512 : (j0 + 1) * 512], pj[0:64, :], b32[0:64, 2:3]
                )
                nc.vector.tensor_scalar_add(
                    viT[:, j1 * 512 : (j1 + 1) * 512], pj[64:128, :], b32[0:64, 2:3]
                )

            def transp(g):
                # viT [64, 512] quarter g -> vi blocks [128, 64] into vaug
                # cols 64-127 via PE transpose.  vaug ones-cols come from the
                # memset filler.
                tr = pp.tile([128, 512], F16, tag="pp", name=f"tr{g}")
                for i in range(4):
                    tb = g * 4 + i
                    nc.tensor.transpose(
                        tr[:, i * 64 : (i + 1) * 64],
                        viT[:, tb * 128 : (tb + 1) * 128],
                        id_ap,
                    )
                dst_ap = vaug[:, g * 512 : (g + 1) * 512].rearrange(
                    "p (t c) -> p t c", c=128
                )[:, :, 64:128]
                src_ap = tr[:, 0:256].rearrange("p (t c) -> p t c", c=H)
                nc.vector.tensor_copy(dst_ap, src_ap)

            def keepalive(n):
                # tiny matmuls that keep the PE HAM activity window busy
                # through DMA-gated stretches of the ramp (a >3.4us PE idle
                # re-throttles the clock to 1.2 GHz)
                def f():
                    for _ in range(n):
                        nc.tensor.matmul(
                            pwu[:, 128:256], lhsT=wu[:], rhs=wu[:],
                            start=True, stop=True,
                        )
                return f

            def proj_h0b(t):
                # unpaired projection of q/k quarter-0 half 1 (256 tokens)
                pj = pp.tile([128, 512], F32, tag="pp", name=f"ph0b{t}")
                src = (q0_in, k0_in)[t]
                w_ap = (wq_ap, wk_ap)[t]
                dst = (qiT, kiT)[t]
                for c in range(EC):
                    nc.tensor.matmul(
                        pj[0:64, 0:256],
                        lhsT=w_ap(c),
                        rhs=src[:, 1, c],
                        start=(c == 0),
                        stop=(c == EC - 1),
                    )
                nc.vector.tensor_scalar_add(
                    dst[0:64, 256:512], pj[0:64, 0:256], b32[0:64, t : t + 1]
                )

            # ---- ramp: quarter-0 halves + unit 0 in 3 chunks, with
            # keepalive batches filling DMA-bound PE idle ----
            proj_qk0(0)
            pt0 = ps.tile([128, 1024], F32, tag="ps", name="sc0_0")

            def sc0(j0, j1):
                nc.tensor.matmul(
                    pt0[:, j0:j1],
                    lhsT=kiT[:, 0:128],
                    rhs=qiT[:, j0:j1],
                    start=True,
                    stop=True,
                )
                nc.scalar.activation(
                    attTs[0][:, j0:j1],
                    pt0[:, j0:j1],
                    mybir.ActivationFunctionType.Exp,
                    scale=INV_SQRT_C,
                )

            sc0(0, 256)
            proj_h0b(0)  # q0b unpaired
            sc0(256, 512)
            proj_one(0, qiT, q_in, 0)  # q1 unpaired: gates unit-0 chunk 3
            sc0(512, 1024)

            F = {
                2: [lambda: proj_h0b(1), lambda: proj_one(1, kiT, k_in, 0)],  # k0b, k1
                5: [lambda: proj_qk_a(1)],  # q2+k2
                6: [lambda: proj_qk_b(1)],
                8: [lambda: proj_v_a(0)],  # v0+v1
                9: [lambda: proj_v_b(0)],
                10: [lambda: proj_qk_a(2)],  # q3+k3
                11: [lambda: proj_qk_b(2)],
                12: [lambda: transp(0)],
                13: [lambda: transp(1), lambda: out_q(0, 0, first=True),
                     lambda: out_q(1, 0)],
                14: [lambda: out_q(2, 0), lambda: out_q(3, 0),
                     lambda: out_q(0, 1, first=True)],
                15: [lambda: proj_v_a(1), lambda: out_q(1, 1),
                     lambda: out_q(5, 1)],  # v2+v3
                16: [lambda: proj_v_b(1), lambda: out_q(4, 0)],
                17: [lambda: out_q(2, 1), lambda: out_q(3, 1), lambda: out_q(5, 0)],
                18: [lambda: transp(2), lambda: out_q(6, 0), lambda: out_q(7, 0)],
                19: [lambda: transp(3), lambda: out_q(4, 1), lambda: out_q(8, 0)],
                20: [lambda: out_q(9, 0), lambda: out_q(6, 1), lambda: out_q(7, 1)],
                21: [lambda: out_q(10, 0), lambda: out_q(11, 0), lambda: out_q(8, 1)],
                22: [lambda: out_q(12, 0), lambda: out_q(13, 0), lambda: out_q(9, 1)],
                23: [lambda: out_q(14, 0), lambda: out_q(15, 0, last=True),
                     lambda: out_q(10, 1)],
                24: [lambda: drain_q(0), lambda: out_q(11, 1), lambda: out_q(12, 1)],
                25: [lambda: out_q(13, 1), lambda: out_q(14, 1),
                     lambda: out_q(15, 1, last=True)],
                26: [lambda: drain_q(1), lambda: out_q(0, 2, first=True),
                     lambda: out_q(1, 2), lambda: out_q(0, 3, first=True)],
                27: [lambda: out_q(2, 2), lambda: out_q(3, 2), lambda: out_q(1, 3),
                     lambda: out_q(2, 3)],
                28: [lambda: out_q(4, 2), lambda: out_q(5, 2), lambda: out_q(3, 3),
                     lambda: out_q(4, 3)],
                29: [lambda: out_q(6, 2), lambda: out_q(7, 2), lambda: out_q(5, 3),
                     lambda: out_q(6, 3)],
                30: [lambda: out_q(8, 2), lambda: out_q(9, 2), lambda: out_q(7, 3),
                     lambda: out_q(8, 3)],
                31: [lambda: out_q(10, 2), lambda: out_q(11, 2), lambda: out_q(9, 3),
                     lambda: out_q(10, 3)],
            }

            def emit_unit(u):
                for f in F.get(u, []):
                    f()
                pt = sc_unit(u)
                exp_unit(u, pt)

            for u in range(1, 20):
                emit_unit(u)

        # pp closed: its banks host the third score rotation tile
        with tc.tile_pool(name="psx", bufs=1, space="PSUM") as psx:
            ps_x_tile[0] = psx
            for u in range(20, 32):
                emit_unit(u)

            # ---- tail: interleave the two remaining chains so q2 closes
            # (and starts draining) while q3's last matmuls still run ----
            out_q(12, 2)
            out_q(11, 3)
            out_q(13, 2)
            out_q(12, 3)
            out_q(14, 2)
            out_q(13, 3)
            out_q(15, 2, last=True)
            out_q(14, 3)
            drain_q(2)
            out_q(15, 3, last=True)
            drain_q(3)

    nc.compile()
    return nc


def _prep_inputs(q, k, v, Wq, bq, Wk, bk, Wv, bv):
    """Host-side layout prep: per-batch transpose + dtype cast + packing."""
    import ml_dtypes

    wa = np.zeros((128, WA_N), dtype=np.float16)
    for t, W in enumerate((Wq, Wk)):
        wa[:, t * WA_K : (t + 1) * WA_K] = (
            np.asarray(W, np.float16).reshape(EC, 128, 64).transpose(1, 0, 2).reshape(128, WA_K)
        )
    wa[0:64, WA_ID : WA_ID + 64] = np.eye(64, dtype=np.float16)
    bpk = np.zeros((128, 32), dtype=np.float16)
    for i, b in enumerate((bq, bk, bv)):
        bpk[:, i] = np.tile(np.asarray(b, dtype=np.float16).reshape(64), 2)
    wvp = np.ascontiguousarray(
        np.asarray(Wv, np.float16).reshape(EC, 128, 64).transpose(1, 0, 2).reshape(128, WA_K)
    )

    def pack_first(x, dt):
        xT = np.asarray(x[:512], dtype=dt).T  # [768, 512]
        return np.ascontiguousarray(xT.reshape(EC, 128, 2, 256).transpose(1, 2, 0, 3))

    def pack_rest(x, dt):
        xT = np.asarray(x[512:], dtype=dt).T  # [768, 1536]
        return np.ascontiguousarray(xT.reshape(EC, 128, 3, 512).transpose(1, 2, 0, 3))

    def pack_full(x, dt):
        xT = np.asarray(x, dtype=dt).T  # [768, 2048]
        return np.ascontiguousarray(xT.reshape(EC, 128, 4, 512).transpose(1, 2, 0, 3))

    f8 = ml_dtypes.float8_e3m4
    in_maps = []
    for i in range(B):
        m = {
            "q0p": pack_first(q[i], f8),
            "k0p": pack_first(k[i], f8),
            "qp": pack_rest(q[i], f8),
            "kp": pack_rest(k[i], f8),
            "vp": pack_full(v[i], np.float16),
            "wa": wa,
            "wv": wvp,
            "bp": bpk,
        }
        in_maps.append(m)
    return in_maps


def run(trace=False, **inputs):
    """Build (cached), run on 8 cores, gather. Returns (out, BassKernelResults)."""
    if "nc" not in _CACHE:
        _CACHE["nc"] = build_program()
    nc = _CACHE["nc"]
    in_maps = _prep_inputs(**{k2: np.asarray(v2) for k2, v2 in inputs.items()})
    res = run_bass_kernel_spmd(nc, in_maps, list(range(B)), trace=trace)
    out = np.stack([np.ascontiguousarray(res.results[i]["outT"].T) for i in range(B)])
    return out.astype(np.float32), res


def kernel(**inputs) -> np.ndarray:
    out, _ = run(trace=False, **inputs)
    return out
